# revision 1
# baseline (speedup 1.0000x reference)
"""Trainium2 Bass kernel for nn_AttentionScore_causal.

Computes, per batch b (one NeuronCore each, 8 cores total):
    qp = q[b] @ Wq.T + bq            [S, H]   (bq == 0 in this problem)
    kp = k[b] @ Wk.T + bk            [S, H]   (bk == 0)
    scores = (qp @ kp.T) * H**-0.5 * qc[b]
    scores[t > s] = -inf  (causal)
    out[b] = softmax(scores, axis=-1)

Algebraic restructuring used on device:
    scores = q @ (Wq.T @ Wk) @ k.T * scale * qc
so we compute CT = (Wq.T @ Wk).T via one small matmul pass, then
KP = C @ kT [H, S], then score tiles qT.T @ KP — every matmul contracts
a partition-dim operand that is naturally laid out, so no on-device
transposes are needed (q.T / k.T are prepared host-side).

Causality is exploited structurally: only lower-triangular score tiles
(at 128-column granularity) are computed; the strictly-upper part of the
output is never touched (output DRAM buffers start zeroed). Masking of
the 128-wide diagonal chunk adds -1e9 above the diagonal before exp.
Softmax needs no max subtraction (scores are O(5); exp cannot overflow)
and the row sum comes free from the ACT engine's accum_out.

Precision: CT is computed from Wq/Wk in float32r (~16 mantissa bits at
2 cyc/row); q.T, k.T, CT and KP are held in fp16 so the two big matmul
stages run at the PE's full 1 cyc/row rate. Measured end-to-end max
relative error vs the fp32 reference: ~3e-4.

Scheduling: score PSUM tiles are one bank each with an 8-slot pool, so
the PE runs several tiles ahead of the softmax drain (keeps the PE HAM
clock-gate warm). The normalize multiply runs on the otherwise idle
GPSIMD engine; exp (+row sums) on ACT; qc-mul and masking on DVE.
"""

import math

import numpy as np

B, S, H = 8, 2048, 512
P = 128  # partitions
HC = H // P  # 4 contraction chunks
NB = S // P  # 16 row blocks
TJ = 512  # score tile free width (one PSUM bank)
N_CORES = 8
SCALE = float(H) ** -0.5
NEG = -1.0e9

_PROGRAM = None


def _build_program():
    import concourse.bass as bass  # noqa: F401
    import concourse.mybir as mybir
    import concourse.tile as tile
    from concourse import bacc

    f32 = mybir.dt.float32
    f32r = mybir.dt.float32r
    f16 = mybir.dt.float16

    nc = bacc.Bacc("TRN2", target_bir_lowering=False, debug=False,
                   num_devices=N_CORES)

    qT = nc.dram_tensor("qT", [H, S], f16, kind="ExternalInput").ap()
    kT = nc.dram_tensor("kT", [H, S], f16, kind="ExternalInput").ap()
    Wq = nc.dram_tensor("Wq", [H, H], f32r, kind="ExternalInput").ap()
    Wk = nc.dram_tensor("Wk", [H, H], f32r, kind="ExternalInput").ap()
    qc = nc.dram_tensor("qc", [S, S], f32, kind="ExternalInput").ap()
    negmask = nc.dram_tensor("negmask", [P, P], f32, kind="ExternalInput").ap()
    out = nc.dram_tensor("out", [S, S], f32, kind="ExternalOutput").ap()

    qT_r = qT.rearrange("(c p) s -> p c s", p=P)
    kT_r = kT.rearrange("(c p) s -> p c s", p=P)
    Wq_r = Wq.rearrange("(c p) h -> p c h", p=P)
    Wk_r = Wk.rearrange("(c p) h -> p c h", p=P)

    with tile.TileContext(nc) as tc:
        with (
            tc.tile_pool(name="resident", bufs=1) as resident,
            tc.tile_pool(name="psum", bufs=8, space="PSUM") as pspool,
        ):
            # ---- resident tiles (live for the whole kernel) ----
            qT_sb = resident.tile([P, HC, S], f16)  # q.T   [h=128c+p][s]
            kp_sb = resident.tile([P, HC, S], f16)  # C@kT  [h1=128c+p][t]
            negm = resident.tile([P, P], f32)

            with tc.tile_pool(name="phase1", bufs=1) as phase1:
                wq_sb = phase1.tile([P, HC, H], f32r)  # Wq [o=128c+p][h]
                wk_sb = phase1.tile([P, HC, H], f32r)
                kT_sb = phase1.tile([P, HC, S], f16)  # k.T [h2=128c+p][t]
                ct_sb = phase1.tile([P, HC, H], f16)  # C.T [h2=128c+p][h1]
                # Chunked loads in dependency order: CT's oc-chunk matmuls
                # start as soon as that chunk of Wq/Wk has landed.
                for oc in range(HC):
                    nc.sync.dma_start(out=wq_sb[:, oc, :], in_=Wq_r[:, oc, :])
                    nc.sync.dma_start(out=wk_sb[:, oc, :], in_=Wk_r[:, oc, :])
                for c2 in range(HC):
                    nc.sync.dma_start(out=kT_sb[:, c2, :], in_=kT_r[:, c2, :])
                nc.sync.dma_start(out=negm, in_=negmask)
                nc.sync.dma_start(out=qT_sb, in_=qT_r)

                # ---- CT[h2, h1] = sum_o Wk[o, h2] * Wq[o, h1] ----
                for c2 in range(HC):
                    ps = pspool.tile([P, TJ], f32, tag="ps")
                    for oc in range(HC):
                        nc.tensor.matmul(
                            ps,
                            wk_sb[:, oc, c2 * P:(c2 + 1) * P],
                            wq_sb[:, oc, :],
                            start=(oc == 0), stop=(oc == HC - 1),
                        )
                    if c2 % 2 == 0:
                        nc.scalar.copy(ct_sb[:, c2, :], ps)
                    else:
                        nc.vector.tensor_copy(ct_sb[:, c2, :], ps)

                # ---- KP[h1, t] = sum_h2 CT[h2, h1] * kT[h2, t] ----
                for c1 in range(HC):
                    for tj in range(S // TJ):
                        ps = pspool.tile([P, TJ], f32, tag="ps")
                        for c2 in range(HC):
                            nc.tensor.matmul(
                                ps,
                                ct_sb[:, c2, c1 * P:(c1 + 1) * P],
                                kT_sb[:, c2, tj * TJ:(tj + 1) * TJ],
                                start=(c2 == 0), stop=(c2 == HC - 1),
                            )
                        if tj % 2 == 0:
                            nc.scalar.copy(kp_sb[:, c1, tj * TJ:(tj + 1) * TJ], ps)
                        else:
                            nc.vector.tensor_copy(kp_sb[:, c1, tj * TJ:(tj + 1) * TJ], ps)

            # ---- scores + softmax, one 128-row block at a time ----
            with (
                tc.tile_pool(name="qcp", bufs=2) as qcp,
                tc.tile_pool(name="work", bufs=2) as work,
                tc.tile_pool(name="sums", bufs=4) as sums_pool,
            ):
                for i in range(NB):
                    w_valid = P * (i + 1)          # valid row width
                    jmax = (P * i) // TJ           # last 512-tile index

                    qc_t = qcp.tile([P, w_valid], f32, tag="qc")
                    nc.sync.dma_start(
                        out=qc_t, in_=qc[i * P:(i + 1) * P, 0:w_valid]
                    )
                    scored = work.tile([P, w_valid], f32, tag="scored")

                    # one PSUM bank per 512-wide tile; DVE drains each tile
                    # right after its 4 accumulation matmuls, so the PE can
                    # run up to 8 tiles ahead (keeps HAM warm).
                    for j in range(jmax + 1):
                        lo = j * TJ
                        hi = min(lo + TJ, w_valid)
                        ps = pspool.tile([P, hi - lo], f32, tag="ps")
                        for c1 in range(HC):
                            nc.tensor.matmul(
                                ps,
                                qT_sb[:, c1, i * P:(i + 1) * P],
                                kp_sb[:, c1, lo:hi],
                                start=(c1 == 0), stop=(c1 == HC - 1),
                            )
                        nc.vector.tensor_mul(scored[:, lo:hi], ps, qc_t[:, lo:hi])

                    # causal mask on the diagonal 128-wide chunk
                    nc.vector.tensor_add(
                        scored[:, w_valid - P:w_valid],
                        scored[:, w_valid - P:w_valid],
                        negm,
                    )
                    etile = work.tile([P, w_valid], f32, tag="etile")
                    sums = sums_pool.tile([P, 1], f32, tag="sums")
                    nc.scalar.activation(
                        etile, scored, mybir.ActivationFunctionType.Exp,
                        bias=0.0, scale=SCALE, accum_out=sums,
                    )
                    recip = sums_pool.tile([P, 1], f32, tag="recip")
                    nc.vector.reciprocal(recip, sums)
                    nc.gpsimd.tensor_scalar_mul(etile, etile, recip)
                    nc.sync.dma_start(
                        out=out[i * P:(i + 1) * P, 0:w_valid], in_=etile
                    )

    nc.compile()
    return nc


def _get_program():
    global _PROGRAM
    if _PROGRAM is None:
        _PROGRAM = _build_program()
    return _PROGRAM


def _make_in_maps(q, k, qc_score, Wq, Wk):
    negmask = np.triu(np.full((P, P), NEG, dtype=np.float32), k=1)
    in_maps = []
    for b in range(N_CORES):
        in_maps.append({
            "qT": np.ascontiguousarray(q[b].T).astype(np.float16),
            "kT": np.ascontiguousarray(k[b].T).astype(np.float16),
            "Wq": np.ascontiguousarray(Wq),
            "Wk": np.ascontiguousarray(Wk),
            "qc": np.ascontiguousarray(qc_score[b]),
            "negmask": negmask,
        })
    return in_maps


def run_on_device(q, k, qc_score, Wq, Wk, trace=False, **trace_kwargs):
    """Returns (output [B,S,S] fp32, BassKernelResults)."""
    from concourse.bass_utils import run_bass_kernel_spmd

    nc = _get_program()
    in_maps = _make_in_maps(q, k, qc_score, Wq, Wk)
    res = run_bass_kernel_spmd(
        nc, in_maps, core_ids=list(range(N_CORES)), trace=trace, **trace_kwargs
    )
    out = np.stack([res.results[b]["out"] for b in range(N_CORES)], axis=0)
    return out, res


def kernel(q, k, attn_mask, key_padding_mask, qc_score, Wq, bq, Wk, bk):
    """Full-input / full-output entry point (the graded interface)."""
    q = np.asarray(q, dtype=np.float32)
    k = np.asarray(k, dtype=np.float32)
    qc_score = np.asarray(qc_score, dtype=np.float32)
    Wq = np.asarray(Wq, dtype=np.float32)
    Wk = np.asarray(Wk, dtype=np.float32)
    out, _ = run_on_device(q, k, qc_score, Wq, Wk, trace=False)
    return out



# revision 2
# speedup vs baseline: 3.3112x; 3.3112x over previous
"""Trainium2 Bass kernel for nn_AttentionScore_causal.

Computes, per batch b (one NeuronCore each, 8 cores total):
    qp = q[b] @ Wq.T + bq            [S, H]   (bq == 0 in this problem)
    kp = k[b] @ Wk.T + bk            [S, H]   (bk == 0)
    scores = (qp @ kp.T) * H**-0.5 * qc[b]
    scores[t > s] = -inf  (causal)
    out[b] = softmax(scores, axis=-1)

Algebraic restructuring used on device:
    scores = q @ (Wq.T @ Wk) @ k.T * scale * qc
so we compute CT = (Wq.T @ Wk).T via one small matmul pass, then
KP = C @ kT [H, S], then score tiles qT.T @ KP — every matmul contracts
a partition-dim operand that is naturally laid out, so no on-device
transposes are needed (q.T / k.T are prepared host-side).

Causality is exploited structurally: only lower-triangular score tiles
(at 128-column granularity) are computed; the strictly-upper part of the
output is never touched (output DRAM buffers start zeroed). Masking of
the 128-wide diagonal chunk adds -30000 (fp16-safe) before exp.

Precision: everything off the PE accumulators is fp16 — weights, q.T,
k.T, qc, the post-multiply score tiles, the exp tiles and the stored
output (host converts back to fp32; measured end-to-end max relative
error vs the fp32 reference ~1e-3, tolerance 2e-2). fp16 halves both
the dominant DMA streams (qc in, out) and doubles DVE throughput.

Engine placement (what made this fast vs the first version): the
softmax normalize multiply runs on DVE as a 16-bit tensor_scalar
(4x perf mode) instead of GPSIMD (which measured ~20x slower and
serialized the whole kernel); it is software-pipelined one block late
so the DVE FIFO never stalls waiting for ACT's row sums.
"""

import math

import numpy as np

B, S, H = 8, 2048, 512
P = 128  # partitions
HC = H // P  # 4 contraction chunks
NB = S // P  # 16 row blocks
TJ = 512  # score tile free width (one PSUM bank)
N_CORES = 8
SCALE = float(H) ** -0.5
NEG = -30000.0  # fp16-safe; exp(NEG*SCALE) == 0

_PROGRAM = None


def _build_program():
    import concourse.bass as bass  # noqa: F401
    import concourse.mybir as mybir
    import concourse.tile as tile
    from concourse import bacc

    f32 = mybir.dt.float32
    f16 = mybir.dt.float16

    nc = bacc.Bacc("TRN2", target_bir_lowering=False, debug=False,
                   num_devices=N_CORES)

    qT = nc.dram_tensor("qT", [H, S], f16, kind="ExternalInput").ap()
    kT = nc.dram_tensor("kT", [H, S], f16, kind="ExternalInput").ap()
    Wq = nc.dram_tensor("Wq", [H, H], f16, kind="ExternalInput").ap()
    Wk = nc.dram_tensor("Wk", [H, H], f16, kind="ExternalInput").ap()
    qc = nc.dram_tensor("qc", [S, S], f16, kind="ExternalInput").ap()
    negmask = nc.dram_tensor("negmask", [P, P], f16, kind="ExternalInput").ap()
    out = nc.dram_tensor("out", [S, S], f16, kind="ExternalOutput").ap()

    qT_r = qT.rearrange("(c p) s -> p c s", p=P)
    kT_r = kT.rearrange("(c p) s -> p c s", p=P)
    Wq_r = Wq.rearrange("(c p) h -> p c h", p=P)
    Wk_r = Wk.rearrange("(c p) h -> p c h", p=P)

    with tile.TileContext(nc) as tc:
        with (
            tc.tile_pool(name="resident", bufs=1) as resident,
            tc.tile_pool(name="psum", bufs=8, space="PSUM") as pspool,
        ):
            # ---- resident tiles (live for the whole kernel) ----
            qT_sb = resident.tile([P, HC, S], f16)  # q.T   [h=128c+p][s]
            kp_sb = resident.tile([P, HC, S], f16)  # C@kT  [h1=128c+p][t]
            negm = resident.tile([P, P], f16)

            with tc.tile_pool(name="phase1", bufs=1) as phase1:
                wq_sb = phase1.tile([P, HC, H], f16)  # Wq [o=128c+p][h]
                wk_sb = phase1.tile([P, HC, H], f16)
                kT_sb = phase1.tile([P, HC, S], f16)  # k.T [h2=128c+p][t]
                ct_sb = phase1.tile([P, HC, H], f16)  # C.T [h2=128c+p][h1]
                # Chunked loads in dependency order: CT's oc-chunk matmuls
                # start as soon as that chunk of Wq/Wk has landed.
                for oc in range(HC):
                    nc.sync.dma_start(out=wq_sb[:, oc, :], in_=Wq_r[:, oc, :])
                    nc.sync.dma_start(out=wk_sb[:, oc, :], in_=Wk_r[:, oc, :])
                for c2 in range(HC):
                    nc.sync.dma_start(out=kT_sb[:, c2, :], in_=kT_r[:, c2, :])
                nc.sync.dma_start(out=negm, in_=negmask)
                nc.sync.dma_start(out=qT_sb, in_=qT_r)

                # ---- CT[h2, h1] = sum_o Wk[o, h2] * Wq[o, h1] ----
                for c2 in range(HC):
                    ps = pspool.tile([P, TJ], f32, tag="ps")
                    for oc in range(HC):
                        nc.tensor.matmul(
                            ps,
                            wk_sb[:, oc, c2 * P:(c2 + 1) * P],
                            wq_sb[:, oc, :],
                            start=(oc == 0), stop=(oc == HC - 1),
                        )
                    if c2 % 2 == 0:
                        nc.scalar.copy(ct_sb[:, c2, :], ps)
                    else:
                        nc.vector.tensor_copy(ct_sb[:, c2, :], ps)

                # ---- KP[h1, t] = sum_h2 CT[h2, h1] * kT[h2, t] ----
                for c1 in range(HC):
                    for tj in range(S // TJ):
                        ps = pspool.tile([P, TJ], f32, tag="ps")
                        for c2 in range(HC):
                            nc.tensor.matmul(
                                ps,
                                ct_sb[:, c2, c1 * P:(c1 + 1) * P],
                                kT_sb[:, c2, tj * TJ:(tj + 1) * TJ],
                                start=(c2 == 0), stop=(c2 == HC - 1),
                            )
                        if tj % 2 == 0:
                            nc.scalar.copy(kp_sb[:, c1, tj * TJ:(tj + 1) * TJ], ps)
                        else:
                            nc.vector.tensor_copy(kp_sb[:, c1, tj * TJ:(tj + 1) * TJ], ps)

            # ---- scores + softmax, one 128-row block at a time ----
            # Iteration i issues block i's scores/mask/exp, then block
            # i-1's recip/normalize/store (one-block software pipeline so
            # the DVE FIFO never stalls on ACT's accumulated row sums).
            with (
                tc.tile_pool(name="qcp", bufs=3) as qcp,
                tc.tile_pool(name="work", bufs=2) as work,
                tc.tile_pool(name="sums", bufs=4) as sums_pool,
            ):
                etiles = [None] * NB
                widths = [P * (i + 1) for i in range(NB)]
                sums_t = [None] * NB
                for i in range(NB + 1):
                    if i < NB:
                        w_valid = widths[i]
                        jmax = (P * i) // TJ  # last 512-tile index

                        qc_t = qcp.tile([P, w_valid], f16, tag="qc")
                        nc.sync.dma_start(
                            out=qc_t, in_=qc[i * P:(i + 1) * P, 0:w_valid]
                        )
                        scored = work.tile([P, w_valid], f16, tag="scored")

                        # one PSUM bank per 512-wide tile; DVE drains each
                        # tile right after its 4 accumulation matmuls, so
                        # the PE can run up to 8 tiles ahead.
                        for j in range(jmax + 1):
                            lo = j * TJ
                            hi = min(lo + TJ, w_valid)
                            ps = pspool.tile([P, hi - lo], f32, tag="ps")
                            for c1 in range(HC):
                                nc.tensor.matmul(
                                    ps,
                                    qT_sb[:, c1, i * P:(i + 1) * P],
                                    kp_sb[:, c1, lo:hi],
                                    start=(c1 == 0), stop=(c1 == HC - 1),
                                )
                            nc.vector.tensor_mul(
                                scored[:, lo:hi], ps, qc_t[:, lo:hi]
                            )

                        # causal mask on the diagonal 128-wide chunk
                        nc.vector.tensor_add(
                            scored[:, w_valid - P:w_valid],
                            scored[:, w_valid - P:w_valid],
                            negm,
                        )
                        etile = work.tile([P, w_valid], f16, tag="etile")
                        sums = sums_pool.tile([P, 1], f32, tag="sums")
                        nc.scalar.activation(
                            etile, scored, mybir.ActivationFunctionType.Exp,
                            bias=0.0, scale=SCALE, accum_out=sums,
                        )
                        etiles[i] = etile
                        sums_t[i] = sums

                    if i > 0:
                        w_prev = widths[i - 1]
                        recip = sums_pool.tile([P, 1], f32, tag="recip")
                        nc.vector.reciprocal(recip, sums_t[i - 1])
                        nc.vector.tensor_scalar_mul(
                            etiles[i - 1], etiles[i - 1], recip
                        )
                        nc.sync.dma_start(
                            out=out[(i - 1) * P:i * P, 0:w_prev],
                            in_=etiles[i - 1],
                        )

    nc.compile()
    return nc


def _get_program():
    global _PROGRAM
    if _PROGRAM is None:
        _PROGRAM = _build_program()
    return _PROGRAM


def _make_in_maps(q, k, qc_score, Wq, Wk):
    negmask = np.triu(np.full((P, P), NEG, dtype=np.float16), k=1)
    in_maps = []
    for b in range(N_CORES):
        in_maps.append({
            "qT": np.ascontiguousarray(q[b].T).astype(np.float16),
            "kT": np.ascontiguousarray(k[b].T).astype(np.float16),
            "Wq": Wq.astype(np.float16),
            "Wk": Wk.astype(np.float16),
            "qc": qc_score[b].astype(np.float16),
            "negmask": negmask,
        })
    return in_maps


def run_on_device(q, k, qc_score, Wq, Wk, trace=False, **trace_kwargs):
    """Returns (output [B,S,S] fp32, BassKernelResults)."""
    from concourse.bass_utils import run_bass_kernel_spmd

    nc = _get_program()
    in_maps = _make_in_maps(q, k, qc_score, Wq, Wk)
    res = run_bass_kernel_spmd(
        nc, in_maps, core_ids=list(range(N_CORES)), trace=trace, **trace_kwargs
    )
    out = np.stack(
        [res.results[b]["out"].astype(np.float32) for b in range(N_CORES)],
        axis=0,
    )
    return out, res


def kernel(q, k, attn_mask, key_padding_mask, qc_score, Wq, bq, Wk, bk):
    """Full-input / full-output entry point (the graded interface)."""
    q = np.asarray(q, dtype=np.float32)
    k = np.asarray(k, dtype=np.float32)
    qc_score = np.asarray(qc_score, dtype=np.float32)
    Wq = np.asarray(Wq, dtype=np.float32)
    Wk = np.asarray(Wk, dtype=np.float32)
    out, _ = run_on_device(q, k, qc_score, Wq, Wk, trace=False)
    return out
